# revision 6
# baseline (speedup 1.0000x reference)
"""Trainium2 Bass kernel for DSAM-style strip-pooling attention recalibration.

Math (reference):
    S_h = mean(x, axis=W); S_v = mean(x, axis=H)
    F   = wh*S_h + wv*S_v                      # broadcast (B,C,H,W)
    Z   = relu(bn(w1 @ F)); A = gelu(w2 @ Z)
    out = x + ls * (x * A)

w1 is linear, so w1 @ (wh*S_h + wv*S_v) splits into per-row / per-column
16-vectors Ph[b,:,h], Pv[b,:,w] with the BN affine folded into the
weights; the broadcast F tensor is never materialized:
    t = relu(Ph[:,h] + Pv[:,w]);  A = gelu(w2 @ t);  out = x*(1 + ls*A)

Sharding: H split across 8 cores (32 rows each). Row sums are local;
Pv partials are summed with ONE (16 x 1024) AllReduce covering all 4
batches (per-batch ARs serialize on the CC stream and their trigger
DMAs cascade behind compute queues).

Design notes (HW-measured rates, not cost-model):
  - x staged fp16 (host cast): 16 MB/core reads, fully SBUF-resident,
    read exactly once. y written f32 (keeps the ls*A*x delta at full
    fidelity; fp16 x costs ~2.1e-4 rel err vs the 2e-2 gate).
  - All x loads enqueued up front on the sync DGE queue.
  - AR critical path rides quiet queues: PSUM-half bounce on ACT,
    psum+sbuf fold on Pool, pv DRAM round-trip on gpsimd SWDGE, so no
    compute queue can delay the collective. A dummy AllReduce issued
    first eats the cross-core barrier/firmware spin-up.
  - Emission is all-A then all-C: pass A (loads, Pv matmuls, row sums,
    Ph) is collective-free and fills the barrier window.
  - tensor_scalar (2-operand) hits the 2x DVE mode on HW; 3-operand
    scalar_tensor_tensor and tensor_tensor with f32 out run 1x. So:
    v = (a*ls)+1 on DVE at 2x, y = x*v split DVE/Pool, t-build relu
    split ACT/Pool.
"""

import functools
import numpy as np

B, C, H, W = 4, 256, 256, 256
CR = 16
N_CORES = 8
H_SH = H // N_CORES          # 32 h-rows per core
HB = 8                       # h-rows per tile
NHB = H_SH // HB             # 4 tile-blocks per core
BN_EPS = 1e-5
NCH = C // 128               # 2 partition chunks of the channel dim
HWH = 1024                   # half-tile free size


@functools.lru_cache(maxsize=1)
def _build():
    import concourse.bacc as bacc
    import concourse.mybir as mybir
    import concourse.tile as tile

    f32 = mybir.dt.float32
    f16 = mybir.dt.float16
    AF = mybir.ActivationFunctionType
    ALU = mybir.AluOpType

    nc = bacc.Bacc("TRN2", target_bir_lowering=False, debug=False,
                   num_devices=N_CORES)

    x_d = nc.dram_tensor("x", [B, C, H_SH, W], f16, kind="ExternalInput")
    w1v_d = nc.dram_tensor("w1v", [C, CR], f16, kind="ExternalInput")
    w1h_d = nc.dram_tensor("w1h", [C, CR], f32, kind="ExternalInput")
    w2t_d = nc.dram_tensor("w2t", [CR, C], f16, kind="ExternalInput")
    gb_d = nc.dram_tensor("gb", [CR, 1], f32, kind="ExternalInput")
    ls_d = nc.dram_tensor("ls", [C, 1], f32, kind="ExternalInput")
    y_d = nc.dram_tensor("y", [B, C, H_SH, W], f32, kind="ExternalOutput")

    with tile.TileContext(nc) as tc:
        with (
            tc.tile_pool(name="consts", bufs=1) as consts,
            tc.tile_pool(name="persist", bufs=1) as persist,
            tc.tile_pool(name="dram", bufs=1, space="DRAM") as dram,
            tc.tile_pool(name="xres", bufs=1) as xres,
            tc.tile_pool(name="yb", bufs=3) as y_pool,
            tc.tile_pool(name="tb", bufs=3) as t_pool,
            tc.tile_pool(name="ab", bufs=2) as a_pool,
            tc.tile_pool(name="vb", bufs=2) as v_pool,
            tc.tile_pool(name="psPv", bufs=2, space="PSUM") as psPv,
            tc.tile_pool(name="psPh", bufs=2, space="PSUM") as psPh,
            tc.tile_pool(name="psC", bufs=2, space="PSUM") as psC,
        ):
            w1v_sb = consts.tile([128, NCH * CR], f16)
            w1h_sb = consts.tile([128, NCH * CR], f32)
            w2t_sb = consts.tile([CR, C], f16)
            gb_sb = consts.tile([CR, 1], f32)
            ls_sb = consts.tile([128, NCH], f32)
            for ch in range(NCH):
                c0 = ch * 128
                nc.sync.dma_start(w1v_sb[:, ch * CR:(ch + 1) * CR],
                                  w1v_d[c0:c0 + 128, :])
                nc.sync.dma_start(w1h_sb[:, ch * CR:(ch + 1) * CR],
                                  w1h_d[c0:c0 + 128, :])
                nc.sync.dma_start(ls_sb[:, ch:ch + 1], ls_d[c0:c0 + 128, :])
            nc.sync.dma_start(w2t_sb[:], w2t_d[:, :])
            nc.sync.dma_start(gb_sb[:], gb_d[:, :])

            s_h_sb = persist.tile([128, NCH * B * H_SH], f32)   # row sums
            ph_sb = persist.tile([CR, B * H_SH], f32)           # Ph + gb
            pv_part_sb = persist.tile([CR, B * W], f32)         # local Pv
            pv_sb = persist.tile([CR, B * W], f32)              # reduced Pv
            pv_bnc = persist.tile([CR, B * W], f32)             # psum bounce

            pv_in_dr = dram.tile([CR, B * W], f32, name="pv_in", tag="pvi")
            pv_out_dr = dram.tile([CR, B * W], f32, name="pv_out", tag="pvo")
            warm_in_dr = dram.tile([CR, 4], f32, name="warm_in", tag="wi")
            warm_out_dr = dram.tile([CR, 4], f32, name="warm_out", tag="wo")

            # dummy AllReduce first on the gpsimd queue: absorbs the
            # cross-core barrier + collective firmware spin-up while
            # pass A streams in.
            nc.gpsimd.collective_compute(
                "AllReduce", ALU.add,
                replica_groups=[list(range(N_CORES))],
                ins=[warm_in_dr[:].opt()],
                outs=[warm_out_dr[:].opt()])

            # enqueue every x-tile load up front (tiles stay resident)
            x_tiles = {}
            for b in range(B):
                for ch in range(NCH):
                    c0 = ch * 128
                    for hb in range(NHB):
                        xt = xres.tile([128, HB * W], f16,
                                       name=f"x{b}_{ch}_{hb}",
                                       tag=f"x{b}_{ch}_{hb}")
                        x_tiles[(b, ch, hb)] = xt
                        nc.sync.dma_start(
                            xt[:],
                            x_d[b, c0:c0 + 128, hb * HB:(hb + 1) * HB, :])

            # ---- pass A: Pv partials (PE) + fold chain per batch ----
            for b in range(B):
                psum_pv = psPv.tile([CR, 512], f32, name=f"psum_pv{b}",
                                    tag="pv")
                first = True
                for ch in range(NCH):
                    for hb in range(NHB):
                        xt = x_tiles[(b, ch, hb)]
                        for k2 in range(2):
                            nc.tensor.matmul(
                                psum_pv[:, :],
                                w1v_sb[:, ch * CR:(ch + 1) * CR],
                                xt[:, k2 * 1024:k2 * 1024 + 512],
                                start=first,
                                stop=(ch == NCH - 1 and hb == NHB - 1
                                      and k2 == 1))
                            first = False
                # bounce both h-halves to SBUF on ACT (gpsimd cannot
                # access PSUM), fold on Pool in SBUF.
                nc.scalar.copy(pv_bnc[:, b * W:(b + 1) * W],
                               psum_pv[:, W:2 * W])
                nc.scalar.copy(pv_part_sb[:, b * W:(b + 1) * W],
                               psum_pv[:, 0:W])
                nc.gpsimd.tensor_tensor(
                    out=pv_part_sb[:, b * W:(b + 1) * W],
                    in0=pv_part_sb[:, b * W:(b + 1) * W],
                    in1=pv_bnc[:, b * W:(b + 1) * W],
                    op=ALU.add)

            # row sums on DVE (free-axis reduce is DVE-only)
            for b in range(B):
                for ch in range(NCH):
                    for hb in range(NHB):
                        xt = x_tiles[(b, ch, hb)]
                        col = ch * B * H_SH + b * H_SH + hb * HB
                        nc.vector.tensor_reduce(
                            out=s_h_sb[:, col:col + HB],
                            in_=xt[:].rearrange("p (h w) -> p h w", w=W),
                            axis=mybir.AxisListType.X, op=ALU.add)

            # ONE AllReduce for all batches; DRAM round-trip on gpsimd
            # SWDGE so no compute queue can delay it.
            nc.gpsimd.dma_start(pv_in_dr[:], pv_part_sb[:])
            nc.gpsimd.collective_compute(
                "AllReduce", ALU.add,
                replica_groups=[list(range(N_CORES))],
                ins=[pv_in_dr[:].opt()],
                outs=[pv_out_dr[:].opt()])
            nc.gpsimd.dma_start(pv_sb[:], pv_out_dr[:])

            # Ph = w1h^T @ s_h (f32) + folded BN bias
            for b in range(B):
                psum_ph = psPh.tile([CR, H_SH], f32, name=f"psum_ph{b}",
                                    tag="ph")
                for ch in range(NCH):
                    col = ch * B * H_SH + b * H_SH
                    nc.tensor.matmul(
                        psum_ph[:, :],
                        w1h_sb[:, ch * CR:(ch + 1) * CR],
                        s_h_sb[:, col:col + H_SH],
                        start=(ch == 0), stop=(ch == NCH - 1))
                nc.scalar.activation(ph_sb[:, b * H_SH:(b + 1) * H_SH],
                                     psum_ph[:, :], AF.Identity,
                                     bias=gb_sb[:, 0:1], scale=1.0)

            # ---- pass C: recalibration per batch ----
            for b in range(B):
                for hb in range(NHB):
                    tb = t_pool.tile([CR, HB * W], f16, name="t_t",
                                     tag="tb")
                    for k in range(HB):
                        col = b * H_SH + hb * HB + k
                        if hb % 2 == 0:
                            # ACT relu: relu(pv + ph)
                            nc.scalar.activation(
                                tb[:, k * W:(k + 1) * W],
                                pv_sb[:, b * W:(b + 1) * W],
                                AF.Relu, bias=ph_sb[:, col:col + 1],
                                scale=1.0)
                        else:
                            # Pool: max(pv + ph, 0)
                            nc.gpsimd.tensor_scalar(
                                out=tb[:, k * W:(k + 1) * W],
                                in0=pv_sb[:, b * W:(b + 1) * W],
                                scalar1=ph_sb[:, col:col + 1], scalar2=0.0,
                                op0=ALU.add, op1=ALU.max)
                    for ch in range(NCH):
                        c0 = ch * 128
                        xt = x_tiles[(b, ch, hb)]
                        yt = y_pool.tile([128, HB * W], f32, name="y_t",
                                         tag="yb")
                        for half in range(2):
                            hof = half * HWH
                            ps = psC.tile([128, HWH], f32, name="ps_t",
                                          tag="ps")
                            for j in range(2):
                                nc.tensor.matmul(
                                    ps[:, j * 512:(j + 1) * 512],
                                    w2t_sb[:, c0:c0 + 128],
                                    tb[:, hof + j * 512:hof + (j + 1) * 512],
                                    start=True, stop=True)
                            ab = a_pool.tile([128, HWH], f16,
                                             name="a_t", tag="ab")
                            nc.scalar.activation(ab[:], ps[:], AF.Gelu)
                            vb = v_pool.tile([128, HWH], f32,
                                             name="v_t", tag="vb")
                            # v = a*ls + 1 (2-op tensor_scalar: 2x mode)
                            nc.vector.tensor_scalar(
                                out=vb[:], in0=ab[:],
                                scalar1=ls_sb[:, ch:ch + 1], scalar2=1.0,
                                op0=ALU.mult, op1=ALU.add)
                            # y = x * v (TT, 1x): split DVE / Pool
                            eng = nc.vector if ch == 0 else nc.gpsimd
                            eng.tensor_tensor(
                                out=yt[:, hof:hof + HWH],
                                in0=xt[:, hof:hof + HWH], in1=vb[:],
                                op=ALU.mult)
                        nc.sync.dma_start(
                            y_d[b, c0:c0 + 128, hb * HB:(hb + 1) * HB, :],
                            yt[:])
    nc.compile()
    return nc


def _prepare(x, w1, w2, bn_gamma, bn_beta, bn_mean, bn_var, weight_h,
             weight_v, layer_scale):
    x = np.asarray(x, dtype=np.float32)
    w1 = np.asarray(w1, dtype=np.float32)
    w2 = np.asarray(w2, dtype=np.float32)
    inv_std = 1.0 / np.sqrt(np.asarray(bn_var, np.float32) + BN_EPS)
    gs = np.asarray(bn_gamma, np.float32) * inv_std
    gb = (np.asarray(bn_beta, np.float32)
          - np.asarray(bn_mean, np.float32) * gs)
    w1s = w1 * gs[:, None]                       # BN scale folded (CR, C)
    wh = float(np.asarray(weight_h).reshape(-1)[0])
    wv = float(np.asarray(weight_v).reshape(-1)[0])
    w1h_t = np.ascontiguousarray(w1s.T * (wh / W)).astype(np.float32)
    w1v_t = np.ascontiguousarray(w1s.T * (wv / H)).astype(np.float16)
    w2t = np.ascontiguousarray(w2.T).astype(np.float16)
    ls = np.ascontiguousarray(
        np.asarray(layer_scale, np.float32).reshape(C, 1))
    gb = np.ascontiguousarray(gb.reshape(CR, 1))
    xh = x.astype(np.float16)
    in_maps = []
    for i in range(N_CORES):
        in_maps.append({
            "x": np.ascontiguousarray(xh[:, :, i * H_SH:(i + 1) * H_SH, :]),
            "w1v": w1v_t, "w1h": w1h_t, "w2t": w2t, "gb": gb, "ls": ls,
        })
    return in_maps


def _run(in_maps, **kwargs):
    from concourse.bass_utils import run_bass_kernel_spmd
    nc = _build()
    return run_bass_kernel_spmd(nc, in_maps, core_ids=list(range(N_CORES)),
                                **kwargs)


def kernel(x, w1, w2, bn_gamma, bn_beta, bn_mean, bn_var, weight_h,
           weight_v, layer_scale):
    in_maps = _prepare(x, w1, w2, bn_gamma, bn_beta, bn_mean, bn_var,
                       weight_h, weight_v, layer_scale)
    res = _run(in_maps)
    y = np.empty((B, C, H, W), dtype=np.float32)
    for i in range(N_CORES):
        y[:, :, i * H_SH:(i + 1) * H_SH, :] = res.results[i]["y"]
    return y


# revision 7
# speedup vs baseline: 2.3617x; 2.3617x over previous
"""Trainium2 Bass kernel for DSAM-style strip-pooling attention recalibration.

Math (reference):
    S_h = mean(x, axis=W); S_v = mean(x, axis=H)
    F   = wh*S_h + wv*S_v                      # broadcast (B,C,H,W)
    Z   = relu(bn(w1 @ F)); A = gelu(w2 @ Z)
    out = x + ls * (x * A)

w1 is linear, so w1 @ (wh*S_h + wv*S_v) splits into per-row / per-column
16-vectors Ph[b,:,h], Pv[b,:,w] with the BN affine folded into the
weights; the broadcast F tensor is never materialized:
    t = relu(Ph[:,h] + Pv[:,w]);  A = gelu(w2 @ t);  out = x*(1 + ls*A)

Sharding: H split across 8 cores (32 rows each). Row sums are local;
Pv partials are summed with ONE (16 x 1024) AllReduce covering all 4
batches, triggered from the otherwise-idle gpsimd queue.

Design notes (HW-measured rates, not cost-model):
  - x staged fp16 (host cast): 16 MB/core reads, fully SBUF-resident,
    read exactly once. y written f32 (keeps the ls*A*x delta at full
    fidelity; fp16 x costs ~2.1e-4 rel err vs the 2e-2 gate).
  - All x loads enqueued up front on the sync DGE queue; pass A (Pv
    matmuls, row sums, Ph) is collective-free and fills the cross-core
    barrier window.
  - Recalibration y = (a*ls + 1)*x is ONE custom DVE op
    (affine_mul_reduce) per [128, 2048] tile half-pair, replacing a
    tensor_scalar + tensor_tensor pair.
  - gpsimd (Pool) engine is ~10x slower than its cost model on bulk
    elementwise: it only carries the AR trigger chain.
  - PSUM pools are scoped: pass A's Pv/Ph banks are released before
    pass C allocates full [128, 2048] z tiles (whole-tile gelu/AMR).
"""

import functools
import numpy as np

B, C, H, W = 4, 256, 256, 256
CR = 16
N_CORES = 8
H_SH = H // N_CORES          # 32 h-rows per core
HB = 8                       # h-rows per tile
NHB = H_SH // HB             # 4 tile-blocks per core
BN_EPS = 1e-5
NCH = C // 128               # 2 partition chunks of the channel dim


@functools.lru_cache(maxsize=1)
def _build():
    import concourse.bacc as bacc
    import concourse.mybir as mybir
    import concourse.tile as tile

    f32 = mybir.dt.float32
    f16 = mybir.dt.float16
    AF = mybir.ActivationFunctionType
    ALU = mybir.AluOpType

    nc = bacc.Bacc("TRN2", target_bir_lowering=False, debug=False,
                   num_devices=N_CORES)

    x_d = nc.dram_tensor("x", [B, C, H_SH, W], f16, kind="ExternalInput")
    w1v_d = nc.dram_tensor("w1v", [C, CR], f16, kind="ExternalInput")
    w1h_d = nc.dram_tensor("w1h", [C, CR], f32, kind="ExternalInput")
    w2t_d = nc.dram_tensor("w2t", [CR, C], f16, kind="ExternalInput")
    gb_d = nc.dram_tensor("gb", [CR, 1], f32, kind="ExternalInput")
    ls_d = nc.dram_tensor("ls", [C, 1], f32, kind="ExternalInput")
    y_d = nc.dram_tensor("y", [B, C, H_SH, W], f32, kind="ExternalOutput")

    with tile.TileContext(nc) as tc:
        with (
            tc.tile_pool(name="consts", bufs=1) as consts,
            tc.tile_pool(name="persist", bufs=1) as persist,
            tc.tile_pool(name="dram", bufs=1, space="DRAM") as dram,
            tc.tile_pool(name="xres", bufs=1) as xres,
            tc.tile_pool(name="yb", bufs=3) as y_pool,
            tc.tile_pool(name="tb", bufs=3) as t_pool,
            tc.tile_pool(name="ab", bufs=2) as a_pool,
        ):
            w1v_sb = consts.tile([128, NCH * CR], f16)
            w1h_sb = consts.tile([128, NCH * CR], f32)
            w2t_sb = consts.tile([CR, C], f16)
            gb_sb = consts.tile([CR, 1], f32)
            ls_sb = consts.tile([128, NCH], f32)
            for ch in range(NCH):
                c0 = ch * 128
                nc.sync.dma_start(w1v_sb[:, ch * CR:(ch + 1) * CR],
                                  w1v_d[c0:c0 + 128, :])
                nc.sync.dma_start(w1h_sb[:, ch * CR:(ch + 1) * CR],
                                  w1h_d[c0:c0 + 128, :])
                nc.sync.dma_start(ls_sb[:, ch:ch + 1], ls_d[c0:c0 + 128, :])
            nc.sync.dma_start(w2t_sb[:], w2t_d[:, :])
            nc.sync.dma_start(gb_sb[:], gb_d[:, :])

            s_h_sb = persist.tile([128, NCH * B * H_SH], f32)   # row sums
            ph_sb = persist.tile([CR, B * H_SH], f32)           # Ph + gb
            pv_part_sb = persist.tile([CR, B * W], f32)         # local Pv
            pv_sb = persist.tile([CR, B * W], f32)              # reduced Pv
            pv_bnc = persist.tile([CR, B * W], f32)             # psum bounce
            acc_scr = persist.tile([128, 1], f32)               # AMR sink

            pv_in_dr = dram.tile([CR, B * W], f32, name="pv_in", tag="pvi")
            pv_out_dr = dram.tile([CR, B * W], f32, name="pv_out", tag="pvo")

            # enqueue every x-tile load up front (tiles stay resident)
            x_tiles = {}
            for b in range(B):
                for ch in range(NCH):
                    c0 = ch * 128
                    for hb in range(NHB):
                        xt = xres.tile([128, HB * W], f16,
                                       name=f"x{b}_{ch}_{hb}",
                                       tag=f"x{b}_{ch}_{hb}")
                        x_tiles[(b, ch, hb)] = xt
                        nc.sync.dma_start(
                            xt[:],
                            x_d[b, c0:c0 + 128, hb * HB:(hb + 1) * HB, :])

            # ---- pass A (collective-free, fills the barrier window) ----
            psA_cm = tc.tile_pool(name="psPv", bufs=2, space="PSUM")
            psA = psA_cm.__enter__()
            psH_cm = tc.tile_pool(name="psPh", bufs=2, space="PSUM")
            psH = psH_cm.__enter__()

            for b in range(B):
                psum_pv = psA.tile([CR, 512], f32, name=f"psum_pv{b}",
                                   tag="pv")
                first = True
                for ch in range(NCH):
                    for hb in range(NHB):
                        xt = x_tiles[(b, ch, hb)]
                        for k2 in range(2):
                            nc.tensor.matmul(
                                psum_pv[:, :],
                                w1v_sb[:, ch * CR:(ch + 1) * CR],
                                xt[:, k2 * 1024:k2 * 1024 + 512],
                                start=first,
                                stop=(ch == NCH - 1 and hb == NHB - 1
                                      and k2 == 1))
                            first = False
                # fold even/odd h halves: ACT bounces odd half to SBUF,
                # DVE adds (one PSUM operand allowed).
                nc.scalar.copy(pv_bnc[:, b * W:(b + 1) * W],
                               psum_pv[:, W:2 * W])
                nc.vector.tensor_tensor(
                    out=pv_part_sb[:, b * W:(b + 1) * W],
                    in0=psum_pv[:, 0:W],
                    in1=pv_bnc[:, b * W:(b + 1) * W],
                    op=ALU.add)

            # ONE AllReduce for all batches; DRAM round-trip + trigger
            # ride the otherwise-idle gpsimd queue.
            nc.gpsimd.dma_start(pv_in_dr[:], pv_part_sb[:])
            nc.gpsimd.collective_compute(
                "AllReduce", ALU.add,
                replica_groups=[list(range(N_CORES))],
                ins=[pv_in_dr[:].opt()],
                outs=[pv_out_dr[:].opt()])
            nc.gpsimd.dma_start(pv_sb[:], pv_out_dr[:])

            # row sums on DVE (free-axis reduce is DVE-only)
            for b in range(B):
                for ch in range(NCH):
                    for hb in range(NHB):
                        xt = x_tiles[(b, ch, hb)]
                        col = ch * B * H_SH + b * H_SH + hb * HB
                        nc.vector.tensor_reduce(
                            out=s_h_sb[:, col:col + HB],
                            in_=xt[:].rearrange("p (h w) -> p h w", w=W),
                            axis=mybir.AxisListType.X, op=ALU.add)

            # Ph = w1h^T @ s_h (f32) + folded BN bias
            for b in range(B):
                psum_ph = psH.tile([CR, H_SH], f32, name=f"psum_ph{b}",
                                   tag="ph")
                for ch in range(NCH):
                    col = ch * B * H_SH + b * H_SH
                    nc.tensor.matmul(
                        psum_ph[:, :],
                        w1h_sb[:, ch * CR:(ch + 1) * CR],
                        s_h_sb[:, col:col + H_SH],
                        start=(ch == 0), stop=(ch == NCH - 1))
                nc.scalar.activation(ph_sb[:, b * H_SH:(b + 1) * H_SH],
                                     psum_ph[:, :], AF.Identity,
                                     bias=gb_sb[:, 0:1], scale=1.0)

            psH_cm.__exit__(None, None, None)
            psA_cm.__exit__(None, None, None)

            # ---- pass C: recalibration per batch ----
            psC_cm = tc.tile_pool(name="psC", bufs=2, space="PSUM")
            psC = psC_cm.__enter__()
            for b in range(B):
                for hb in range(NHB):
                    tb = t_pool.tile([CR, HB * W], f16, name="t_t",
                                     tag="tb")
                    for k in range(HB):
                        col = b * H_SH + hb * HB + k
                        if k % 4 != 0:
                            # ACT relu: relu(pv + ph)    (3/4 of rows)
                            nc.scalar.activation(
                                tb[:, k * W:(k + 1) * W],
                                pv_sb[:, b * W:(b + 1) * W],
                                AF.Relu, bias=ph_sb[:, col:col + 1],
                                scale=1.0)
                        else:
                            # DVE: max(pv + ph, 0)       (1/4 of rows)
                            nc.vector.tensor_scalar(
                                out=tb[:, k * W:(k + 1) * W],
                                in0=pv_sb[:, b * W:(b + 1) * W],
                                scalar1=ph_sb[:, col:col + 1], scalar2=0.0,
                                op0=ALU.add, op1=ALU.max)
                    for ch in range(NCH):
                        c0 = ch * 128
                        xt = x_tiles[(b, ch, hb)]
                        yt = y_pool.tile([128, HB * W], f32, name="y_t",
                                         tag="yb")
                        ps = psC.tile([128, HB * W], f32, name="ps_t",
                                      tag="ps")
                        for j in range(4):
                            nc.tensor.matmul(
                                ps[:, j * 512:(j + 1) * 512],
                                w2t_sb[:, c0:c0 + 128],
                                tb[:, j * 512:(j + 1) * 512],
                                start=True, stop=True)
                        ab = a_pool.tile([128, HB * W], f16,
                                         name="a_t", tag="ab")
                        nc.scalar.activation(ab[:], ps[:], AF.Gelu)
                        # y = (a*ls + 1) * x in ONE custom DVE op
                        nc.vector.affine_mul_reduce(
                            out=yt[:], accum_out=acc_scr[:],
                            in0=ab[:], in1=xt[:],
                            scale=ls_sb[:, ch:ch + 1], bias=1.0)
                        nc.sync.dma_start(
                            y_d[b, c0:c0 + 128, hb * HB:(hb + 1) * HB, :],
                            yt[:])
            psC_cm.__exit__(None, None, None)
    nc.compile()
    return nc


def _prepare(x, w1, w2, bn_gamma, bn_beta, bn_mean, bn_var, weight_h,
             weight_v, layer_scale):
    x = np.asarray(x, dtype=np.float32)
    w1 = np.asarray(w1, dtype=np.float32)
    w2 = np.asarray(w2, dtype=np.float32)
    inv_std = 1.0 / np.sqrt(np.asarray(bn_var, np.float32) + BN_EPS)
    gs = np.asarray(bn_gamma, np.float32) * inv_std
    gb = (np.asarray(bn_beta, np.float32)
          - np.asarray(bn_mean, np.float32) * gs)
    w1s = w1 * gs[:, None]                       # BN scale folded (CR, C)
    wh = float(np.asarray(weight_h).reshape(-1)[0])
    wv = float(np.asarray(weight_v).reshape(-1)[0])
    w1h_t = np.ascontiguousarray(w1s.T * (wh / W)).astype(np.float32)
    w1v_t = np.ascontiguousarray(w1s.T * (wv / H)).astype(np.float16)
    w2t = np.ascontiguousarray(w2.T).astype(np.float16)
    ls = np.ascontiguousarray(
        np.asarray(layer_scale, np.float32).reshape(C, 1))
    gb = np.ascontiguousarray(gb.reshape(CR, 1))
    xh = x.astype(np.float16)
    in_maps = []
    for i in range(N_CORES):
        in_maps.append({
            "x": np.ascontiguousarray(xh[:, :, i * H_SH:(i + 1) * H_SH, :]),
            "w1v": w1v_t, "w1h": w1h_t, "w2t": w2t, "gb": gb, "ls": ls,
        })
    return in_maps


def _run(in_maps, **kwargs):
    from concourse.bass_utils import run_bass_kernel_spmd
    nc = _build()
    return run_bass_kernel_spmd(nc, in_maps, core_ids=list(range(N_CORES)),
                                **kwargs)


def kernel(x, w1, w2, bn_gamma, bn_beta, bn_mean, bn_var, weight_h,
           weight_v, layer_scale):
    in_maps = _prepare(x, w1, w2, bn_gamma, bn_beta, bn_mean, bn_var,
                       weight_h, weight_v, layer_scale)
    res = _run(in_maps)
    y = np.empty((B, C, H, W), dtype=np.float32)
    for i in range(N_CORES):
        y[:, :, i * H_SH:(i + 1) * H_SH, :] = res.results[i]["y"]
    return y
